# revision 10
# baseline (speedup 1.0000x reference)
"""Trainium2 Bass kernel for nn_CrossAttention (b=2, sq=sk=2048, d=1024, h=16).

Wire-optimized sharding: per-call wall clock is dominated by host<->device
transfer over the axon tunnel (~100MB/s, serialized across cores), so every
tensor is shipped exactly once in fp16 with no cross-core replication:
each of the 8 cores owns 2 heads x both batches (a contiguous 128-column
d-slice of q/k/v in natural [s, d] layout — host does only fp16 casts, all
layout work happens on device). The o_proj weight is d-sharded 8-way and the
partial yT[j, b, q] outputs are summed with an on-device ReduceScatter; each
core downloads its disjoint 128-feature slice quantized to int8 with
per-feature-row scales (rel err ~0.4% vs the 2e-2 budget).

Per-core device pipeline:
  qT/kT tiles  = PE transpose of natural q/k tiles      (PE + DVE copy)
  vn          *= km (key mask, multiplicative)          (DVE)
  scoresT[s,q] = sum_d K[s,d] Q[q,d]                    (PE, fp16 -> f32 psum)
  expT         = exp(0.125 * scoresT)                   (ACT, psum->sbuf fp16)
  avT[0:64,q]  = sum_s vn[s,m] expT[s,q]                (PE, accumulated)
  avT[64,q]    = sum_s km[s] expT[s,q]   (denominator)  (PE, accumulated)
  sc[q]        = rs[q] / (avT[64,q] + eps)              (DVE on 1 partition)
  bc[i,q]      = sc[q]                                  (PE K=1 broadcast)
  outT[...]    = avT[i,q] * bc[i,q]                     (DVE -> fp16)
  y_part       = WoT-slice @ outT                       (PE)
  y_rs         = ReduceScatter_add(y_part, ranks 0..7)  (rank keeps jc==rank)
  yq, mx       = int8 quantize with per-row absmax      (DVE + ACT)

Query-mask / fully-masked rows are zeroed by rs, matching the reference's
nan_to_num semantics. Host adds bo and dequantizes during assembly.
"""

import numpy as np

import concourse.mybir as mybir
import concourse.tile as tile
from concourse import bacc
from concourse import bass_utils

FP16 = mybir.dt.float16
F32 = mybir.dt.float32

# full-problem constants
B, SQ, SK, D, H, HD = 2, 2048, 2048, 1024, 16, 64
NCORES = 8
HLOC = H // NCORES  # 2 heads per core
QBLK = 512  # q columns per psum tile
SKT = SK // 128  # 16 s tiles (same count for q and k)

# per-iteration sk-tile chunking: sizes sum to SKT, tags strictly alternate
# so psum-slot reuse distance stays >= 2 across iteration boundaries
CHUNK_PLAN = [(3, "A"), (3, "B"), (3, "A"), (3, "B"), (2, "A"), (2, "B")]


def build_program():
    nih = B * HLOC  # 4 (batch, local-head) pairs
    nqb = SQ // QBLK  # 4 q blocks
    nj = D // 128  # 8 output-feature chunks (ReduceScatter dim)
    nc = bacc.Bacc(
        "TRN2",
        target_bir_lowering=False,
        debug=False,
        enable_asserts=False,
        num_devices=NCORES,
    )

    qn = nc.dram_tensor("qn", [B, SQ, 128], FP16, kind="ExternalInput").ap()
    kn = nc.dram_tensor("kn", [B, SK, 128], FP16, kind="ExternalInput").ap()
    vn = nc.dram_tensor("vn", [B, SK, 128], FP16, kind="ExternalInput").ap()
    km2 = nc.dram_tensor("km2", [128, B * SKT], FP16, kind="ExternalInput").ap()
    ident = nc.dram_tensor("ident", [128, 128], FP16, kind="ExternalInput").ap()
    wot = nc.dram_tensor("wot", [128, D], FP16, kind="ExternalInput").ap()
    rs = nc.dram_tensor("rs", [1, B * SQ], F32, kind="ExternalInput").ap()
    ones = nc.dram_tensor("ones", [1, 64], F32, kind="ExternalInput").ap()
    yq = nc.dram_tensor("yq", [128, B, SQ], mybir.dt.int8, kind="ExternalOutput").ap()
    mxo = nc.dram_tensor("mx", [128, 1], F32, kind="ExternalOutput").ap()

    with tile.TileContext(nc) as tc:
        with (
            tc.tile_pool(name="const", bufs=1) as cpool,
            tc.tile_pool(name="exp", bufs=4) as epool,
            tc.tile_pool(name="drain", bufs=2) as dpool,
            tc.tile_pool(name="pA", bufs=1, space="PSUM") as pA,
            tc.tile_pool(name="pB", bufs=1, space="PSUM") as pB,
            tc.tile_pool(name="pacc", bufs=1, space="PSUM") as pacc,
            tc.tile_pool(name="pbc", bufs=1, space="PSUM") as pbc,
            tc.tile_pool(name="dram", bufs=1, space="DRAM") as dram,
        ):
            qn_sb = cpool.tile([128, B, SKT, 128], FP16)
            kn_sb = cpool.tile([128, B, SKT, 128], FP16)
            vn_sb = cpool.tile([128, B, SKT, 128], FP16)
            km2_sb = cpool.tile([128, B, SKT], FP16)
            ident_sb = cpool.tile([128, 128], FP16)
            qt_sb = cpool.tile([64, nih, SQ], FP16)
            kt_sb = cpool.tile([64, nih, SK], FP16)
            wot_sb = cpool.tile([128, D], FP16)
            rs_sb = cpool.tile([1, B * SQ], F32)
            ones_sb = cpool.tile([1, 64], F32)
            outT_sb = cpool.tile([128, B, SQ], FP16)

            y_part = dram.tile([nj, 128, B, SQ], FP16)
            y_rs = dram.tile([128, B, SQ], FP16)

            for b in range(B):
                nc.sync.dma_start(
                    qn_sb[:, b], qn[b].rearrange("(t p) d -> p t d", p=128)
                )
                nc.sync.dma_start(
                    kn_sb[:, b], kn[b].rearrange("(t p) d -> p t d", p=128)
                )
                nc.sync.dma_start(
                    vn_sb[:, b], vn[b].rearrange("(t p) d -> p t d", p=128)
                )
            nc.sync.dma_start(km2_sb[:], km2.rearrange("p (b t) -> p b t", b=B))
            nc.sync.dma_start(ident_sb[:], ident[:])
            nc.sync.dma_start(wot_sb[:], wot[:])
            nc.sync.dma_start(rs_sb[:], rs[:])
            nc.sync.dma_start(ones_sb[:], ones[:])

            # fold key mask into v (multiplicative)
            for b in range(B):
                for t in range(SKT):
                    nc.vector.tensor_tensor(
                        vn_sb[:, b, t, :],
                        vn_sb[:, b, t, :],
                        km2_sb[:, b, t : t + 1].to_broadcast((128, 128)),
                        mybir.AluOpType.mult,
                    )

            # PE-transpose natural q/k tiles into [hd, s] operand layout
            tpools = (pacc, pbc)
            ttags = ("acc", "bc")
            idx = 0
            for ih in range(nih):
                b, hl = ih // HLOC, ih % HLOC
                for t in range(SKT):
                    for src, dst in ((qn_sb, qt_sb), (kn_sb, kt_sb)):
                        tp = tpools[idx % 2].tile([64, 128], FP16, tag=ttags[idx % 2])
                        idx += 1
                        nc.tensor.transpose(
                            tp[:], src[:, b, t, hl * 64 : hl * 64 + 64], ident_sb[:]
                        )
                        nc.vector.tensor_copy(dst[:, ih, t * 128 : (t + 1) * 128], tp[:])

            av_pss = {}

            def drain_iter(it):
                ih, qb = it
                b = ih // HLOC
                hl = ih % HLOC
                qsl = slice(qb * QBLK, (qb + 1) * QBLK)
                av_sb = dpool.tile([65, QBLK], F32, tag="avsb")
                nc.vector.tensor_copy(av_sb[:], av_pss[it][:])
                sc = dpool.tile([1, QBLK], F32, tag="sc")
                nc.vector.tensor_scalar_add(sc[:], av_sb[64:65, :], 1e-30)
                nc.vector.reciprocal(sc[:], sc[:])
                nc.vector.tensor_mul(
                    sc[:],
                    sc[:],
                    rs_sb[0:1, b * SQ + qb * QBLK : b * SQ + (qb + 1) * QBLK],
                )
                bc_ps = pbc.tile([64, QBLK], F32, tag="bc")
                nc.tensor.matmul(
                    bc_ps[:], lhsT=ones_sb[:], rhs=sc[:], start=True, stop=True
                )
                nc.vector.tensor_tensor(
                    outT_sb[hl * 64 : hl * 64 + 64, b, qsl],
                    av_sb[0:64, :],
                    bc_ps[:],
                    mybir.AluOpType.mult,
                )

            # flat, software-pipelined chunk stream: QK(c+1) is emitted
            # before AV(c) so the in-order PE queue never waits on exp(c)
            chunks = []
            for ih in range(nih):
                for qb in range(nqb):
                    t0 = 0
                    for csz, tag in CHUNK_PLAN:
                        chunks.append((ih, qb, t0, csz, tag))
                        t0 += csz

            def emit_av(item):
                ih, qb, t0, csz, ex = item
                it = (ih, qb)
                b, hl = ih // HLOC, ih % HLOC
                for j in range(csz):
                    t = t0 + j
                    exsl = ex[:, j * QBLK : (j + 1) * QBLK]
                    nc.tensor.matmul(
                        av_pss[it][0:64, :],
                        lhsT=vn_sb[:, b, t, hl * 64 : hl * 64 + 64],
                        rhs=exsl,
                        start=(t == 0),
                        stop=(t == SKT - 1),
                    )
                    nc.tensor.matmul(
                        av_pss[it][64:65, :],
                        lhsT=km2_sb[:, b, t : t + 1],
                        rhs=exsl,
                        start=(t == 0),
                        stop=(t == SKT - 1),
                    )
                if t0 + csz == SKT:
                    drain_iter(it)

            pending = []  # depth-2 queue of chunks awaiting AV
            for ci, (ih, qb, t0, csz, tag) in enumerate(chunks):
                it = (ih, qb)
                if t0 == 0:
                    av_pss[it] = pacc.tile(
                        [65, QBLK], F32, tag="acc", name=f"av_ps{ih}_{qb}"
                    )
                pool = pA if tag == "A" else pB
                qk_ps = pool.tile(
                    [128, csz * QBLK], F32, tag="qk" + tag, name=f"qk_ps{ci}"
                )
                qsl = slice(qb * QBLK, (qb + 1) * QBLK)
                for j in range(csz):
                    t = t0 + j
                    nc.tensor.matmul(
                        qk_ps[:, j * QBLK : (j + 1) * QBLK],
                        lhsT=kt_sb[:, ih, t * 128 : (t + 1) * 128],
                        rhs=qt_sb[:, ih, qsl],
                        start=True,
                        stop=True,
                    )
                if len(pending) == 2:
                    emit_av(pending.pop(0))
                ex = epool.tile([128, csz * QBLK], FP16, tag="exp")
                nc.scalar.activation(
                    ex[:], qk_ps[:], mybir.ActivationFunctionType.Exp, scale=0.125
                )
                pending.append((ih, qb, t0, csz, ex))

            for item in pending:
                emit_av(item)

            # partial o-proj: y_part[jc, j, b, q] = sum_p wot[p, jc*128+j] outT[p, b, q]
            for jc in range(nj):
                for b in range(B):
                    for qb in range(nqb):
                        pool = pA if (jc * B * nqb + b * nqb + qb) % 2 == 0 else pB
                        y_ps = pool.tile(
                            [128, QBLK], F32, tag="qk" + ("A" if pool is pA else "B")
                        )
                        qsl = slice(qb * QBLK, (qb + 1) * QBLK)
                        nc.tensor.matmul(
                            y_ps[:],
                            lhsT=wot_sb[:, jc * 128 : (jc + 1) * 128],
                            rhs=outT_sb[:, b, qsl],
                            start=True,
                            stop=True,
                        )
                        y_sb = dpool.tile([128, QBLK], FP16, tag="y")
                        nc.vector.tensor_copy(y_sb[:], y_ps[:])
                        nc.sync.dma_start(y_part[jc, :, b, qsl], y_sb[:])

            # column-sharded o-proj all-reduce: each rank keeps jc == rank
            nc.gpsimd.collective_compute(
                "ReduceScatter",
                mybir.AluOpType.add,
                replica_groups=[list(range(NCORES))],
                ins=[y_part.opt()],
                outs=[y_rs.opt()],
            )

            # int8 downcast with per-feature-row scales: halves the download
            y_all = cpool.tile([128, B, SQ], FP16)
            nc.sync.dma_start(y_all[:], y_rs[:])
            mx_sb = cpool.tile([128, 1], F32)
            nc.vector.tensor_reduce(
                mx_sb[:],
                y_all[:],
                axis=mybir.AxisListType.XY,
                op=mybir.AluOpType.max,
                apply_absolute_value=True,
            )
            inv_sb = cpool.tile([128, 1], F32)
            nc.vector.tensor_scalar_add(inv_sb[:], mx_sb[:], 1e-30)
            nc.vector.reciprocal(inv_sb[:], inv_sb[:])
            nc.vector.tensor_scalar_mul(inv_sb[:], inv_sb[:], 127.0)
            yq_sb = cpool.tile([128, B, SQ], mybir.dt.int8)
            nc.scalar.activation(
                yq_sb[:], y_all[:], mybir.ActivationFunctionType.Copy, scale=inv_sb[:]
            )
            nc.sync.dma_start(yq[:], yq_sb[:])
            nc.sync.dma_start(mxo[:], mx_sb[:])

    nc.compile()
    return nc


def shard_inputs(query, key, value, key_mask, query_mask, Wo, bo):
    """Full inputs -> per-core input maps. Host does only casts + tiny prep;
    all layout transposes happen on device."""
    km01 = key_mask[:, :, 0] != 0  # [B, SK] bool
    qm01 = query_mask[:, :, 0] != 0  # [B, SQ]
    any_km = km01.any(axis=1)  # [B]

    q16 = query.astype(np.float16)
    k16 = key.astype(np.float16)
    v16 = value.astype(np.float16)
    km2 = np.ascontiguousarray(
        km01.astype(np.float16).reshape(B, SKT, 128).transpose(2, 0, 1)
    ).reshape(128, B * SKT)
    ident = np.eye(128, dtype=np.float16)
    woT = np.ascontiguousarray(Wo.T).astype(np.float16)  # [d, j]
    rs_all = (qm01 & any_km[:, None]).astype(np.float32).reshape(1, B * SQ)
    ones = np.ones((1, 64), np.float32)

    in_maps = []
    for c in range(NCORES):
        dsl = slice(128 * c, 128 * (c + 1))  # heads 2c, 2c+1
        in_maps.append(
            {
                "qn": q16[:, :, dsl],
                "kn": k16[:, :, dsl],
                "vn": v16[:, :, dsl],
                "km2": km2,
                "ident": ident,
                "wot": woT[dsl],
                "rs": rs_all,
                "ones": ones,
            }
        )
    return in_maps


_NC_CACHE = {}


def _get_program():
    if "nc" not in _NC_CACHE:
        _NC_CACHE["nc"] = build_program()
    return _NC_CACHE["nc"]


def kernel(query, key, value, key_mask, query_mask, Wo, bo, _trace=False):
    query = np.asarray(query, dtype=np.float32)
    key = np.asarray(key, dtype=np.float32)
    value = np.asarray(value, dtype=np.float32)
    key_mask = np.asarray(key_mask, dtype=np.int32)
    query_mask = np.asarray(query_mask, dtype=np.int32)
    Wo = np.asarray(Wo, dtype=np.float32)
    bo = np.asarray(bo, dtype=np.float32)

    nc = _get_program()
    in_maps = shard_inputs(query, key, value, key_mask, query_mask, Wo, bo)
    try:
        res = bass_utils.run_bass_kernel_spmd(
            nc, in_maps, core_ids=list(range(NCORES)), trace=_trace
        )
    except ModuleNotFoundError:
        # axon NTFF profile hook unavailable in this container; run untraced
        res = bass_utils.run_bass_kernel_spmd(
            nc, in_maps, core_ids=list(range(NCORES)), trace=False
        )
    kernel.last_results = res

    out = np.empty((B, SQ, D), np.float32)
    for c in range(NCORES):
        ytq = res.results[c]["yq"]  # int8 [128, B, SQ], features 128c..128c+127
        sc = res.results[c]["mx"] * np.float32(1.0 / 127.0)  # [128, 1]
        jsl = slice(128 * c, 128 * (c + 1))
        for g in range(B):
            out[g, :, jsl] = (ytq[:, g, :] * sc).T + bo[jsl]
    return out


# revision 18
# speedup vs baseline: 1.0315x; 1.0315x over previous
"""Trainium2 Bass kernel for nn_CrossAttention (b=2, sq=sk=2048, d=1024, h=16).

Wire-optimized sharding: per-call wall clock is dominated by host<->device
transfer over the axon tunnel (~100MB/s, serialized across cores), so every
tensor is shipped exactly once in fp16 with no cross-core replication:
each of the 8 cores owns 2 heads x both batches (a contiguous 128-column
d-slice of q/k/v in natural [s, d] layout — host does only fp16 casts, all
layout work happens on device). The o_proj weight is d-sharded 8-way and the
partial yT[j, b, q] outputs are summed with an on-device ReduceScatter; each
core downloads its disjoint 128-feature slice quantized to int8 with
per-feature-row scales (rel err ~0.4% vs the 2e-2 budget).

Per-core device pipeline:
  qT/kT tiles  = PE transpose of natural q/k tiles      (PE + DVE copy)
  vn          *= km (key mask, multiplicative)          (DVE)
  scoresT[s,q] = sum_d K[s,d] Q[q,d]                    (PE, fp16 -> f32 psum)
  expT         = exp(0.125 * scoresT)                   (ACT, psum->sbuf fp16)
  avT[0:64,q]  = sum_s vn[s,m] expT[s,q]                (PE, accumulated)
  avT[64,q]    = sum_s km[s] expT[s,q]   (denominator)  (PE, accumulated)
  sc[q]        = rs[q] / (avT[64,q] + eps)              (DVE on 1 partition)
  bc[i,q]      = sc[q]                                  (PE K=1 broadcast)
  outT[...]    = avT[i,q] * bc[i,q]                     (DVE -> fp16)
  y_part       = WoT-slice @ outT                       (PE)
  y_rs         = ReduceScatter_add(y_part, ranks 0..7)  (rank keeps jc==rank)
  yq, mx       = int8 quantize with per-row absmax      (DVE + ACT)

Query-mask / fully-masked rows are zeroed by rs, matching the reference's
nan_to_num semantics. Host adds bo and dequantizes during assembly.
"""

import numpy as np

import concourse.mybir as mybir
import concourse.tile as tile
from concourse import bacc
from concourse import bass_utils

FP16 = mybir.dt.float16
F32 = mybir.dt.float32

# full-problem constants
B, SQ, SK, D, H, HD = 2, 2048, 2048, 1024, 16, 64
NCORES = 8
HLOC = H // NCORES  # 2 heads per core
QBLK = 512  # q columns per psum tile
SKT = SK // 128  # 16 s tiles (same count for q and k)

# per-iteration sk-tile chunking: sizes sum to SKT, tags strictly alternate
# so psum-slot reuse distance stays >= 2 across iteration boundaries
CHUNK_PLAN = [(3, "A"), (3, "B"), (3, "A"), (3, "B"), (2, "A"), (2, "B")]


def build_program():
    nih = B * HLOC  # 4 (batch, local-head) pairs
    nqb = SQ // QBLK  # 4 q blocks
    nj = D // 128  # 8 output-feature chunks (ReduceScatter dim)
    nc = bacc.Bacc(
        "TRN2",
        target_bir_lowering=False,
        debug=False,
        enable_asserts=False,
        num_devices=NCORES,
    )

    # inputs merged into 3 arrays (per-array transfer overhead is ~7ms)
    qkv = nc.dram_tensor("qkv", [3, B, SQ, 128], FP16, kind="ExternalInput").ap()
    # misc16 cols: km2 [0:B*SKT] | ident [B*SKT:B*SKT+128] | wot [B*SKT+128:]
    misc16 = nc.dram_tensor(
        "misc16", [128, B * SKT + 128 + D], FP16, kind="ExternalInput"
    ).ap()
    # misc32 cols: rs [0:B*SQ] | ones [B*SQ:B*SQ+64]
    misc32 = nc.dram_tensor("misc32", [1, B * SQ + 64], F32, kind="ExternalInput").ap()
    yq = nc.dram_tensor("yq", [128, B, SQ], mybir.dt.int8, kind="ExternalOutput").ap()
    mxo = nc.dram_tensor("mx", [128, 1], F32, kind="ExternalOutput").ap()

    with tile.TileContext(nc) as tc:
        with (
            tc.tile_pool(name="const", bufs=1) as cpool,
            tc.tile_pool(name="exp", bufs=4) as epool,
            tc.tile_pool(name="drain", bufs=2) as dpool,
            tc.tile_pool(name="pA", bufs=1, space="PSUM") as pA,
            tc.tile_pool(name="pB", bufs=1, space="PSUM") as pB,
            tc.tile_pool(name="pacc", bufs=1, space="PSUM") as pacc,
            tc.tile_pool(name="pbc", bufs=1, space="PSUM") as pbc,
            tc.tile_pool(name="dram", bufs=1, space="DRAM") as dram,
        ):
            qn_sb = cpool.tile([128, B, SKT, 128], FP16)
            kn_sb = cpool.tile([128, B, SKT, 128], FP16)
            vn_sb = cpool.tile([128, B, SKT, 128], FP16)
            m16_sb = cpool.tile([128, B * SKT + 128 + D], FP16)
            m32_sb = cpool.tile([1, B * SQ + 64], F32)
            qt_sb = cpool.tile([64, nih, SQ], FP16)
            kt_sb = cpool.tile([64, nih, SK], FP16)
            outT_sb = cpool.tile([128, B, SQ], FP16)
            def km2_col(b, t):  # [128, 1] key-mask column for (batch, sk-tile)
                return m16_sb[:, b * SKT + t : b * SKT + t + 1]

            ident_sb = m16_sb[:, B * SKT : B * SKT + 128]
            wot0 = B * SKT + 128  # wot column base inside m16_sb
            rs_sb = m32_sb[:, 0 : B * SQ]
            ones_sb = m32_sb[:, B * SQ : B * SQ + 64]

            y_part = dram.tile([nj, 128, B, SQ], FP16)
            y_rs = dram.tile([128, B, SQ], FP16)

            for b in range(B):
                nc.sync.dma_start(
                    qn_sb[:, b], qkv[0, b].rearrange("(t p) d -> p t d", p=128)
                )
                nc.sync.dma_start(
                    kn_sb[:, b], qkv[1, b].rearrange("(t p) d -> p t d", p=128)
                )
                nc.sync.dma_start(
                    vn_sb[:, b], qkv[2, b].rearrange("(t p) d -> p t d", p=128)
                )
            nc.sync.dma_start(m16_sb[:], misc16[:])
            nc.sync.dma_start(m32_sb[:], misc32[:])

            # fold key mask into v (multiplicative)
            for b in range(B):
                for t in range(SKT):
                    nc.vector.tensor_tensor(
                        vn_sb[:, b, t, :],
                        vn_sb[:, b, t, :],
                        km2_col(b, t).to_broadcast((128, 128)),
                        mybir.AluOpType.mult,
                    )

            # PE-transpose natural q/k tiles into [hd, s] operand layout
            tpools = (pacc, pbc)
            ttags = ("acc", "bc")
            idx = 0
            for ih in range(nih):
                b, hl = ih // HLOC, ih % HLOC
                for t in range(SKT):
                    for src, dst in ((qn_sb, qt_sb), (kn_sb, kt_sb)):
                        tp = tpools[idx % 2].tile([64, 128], FP16, tag=ttags[idx % 2])
                        idx += 1
                        nc.tensor.transpose(
                            tp[:], src[:, b, t, hl * 64 : hl * 64 + 64], ident_sb[:]
                        )
                        nc.vector.tensor_copy(dst[:, ih, t * 128 : (t + 1) * 128], tp[:])

            av_pss = {}

            def drain_iter(it):
                ih, qb = it
                b = ih // HLOC
                hl = ih % HLOC
                qsl = slice(qb * QBLK, (qb + 1) * QBLK)
                av_sb = dpool.tile([65, QBLK], F32, tag="avsb")
                nc.vector.tensor_copy(av_sb[:], av_pss[it][:])
                sc = dpool.tile([1, QBLK], F32, tag="sc")
                nc.vector.tensor_scalar_add(sc[:], av_sb[64:65, :], 1e-30)
                nc.vector.reciprocal(sc[:], sc[:])
                nc.vector.tensor_mul(
                    sc[:],
                    sc[:],
                    rs_sb[0:1, b * SQ + qb * QBLK : b * SQ + (qb + 1) * QBLK],
                )
                bc_ps = pbc.tile([64, QBLK], F32, tag="bc")
                nc.tensor.matmul(
                    bc_ps[:], lhsT=ones_sb[:], rhs=sc[:], start=True, stop=True
                )
                nc.vector.tensor_tensor(
                    outT_sb[hl * 64 : hl * 64 + 64, b, qsl],
                    av_sb[0:64, :],
                    bc_ps[:],
                    mybir.AluOpType.mult,
                )

            # flat, software-pipelined chunk stream: QK(c+1) is emitted
            # before AV(c) so the in-order PE queue never waits on exp(c)
            chunks = []
            for ih in range(nih):
                for qb in range(nqb):
                    t0 = 0
                    for csz, tag in CHUNK_PLAN:
                        chunks.append((ih, qb, t0, csz, tag))
                        t0 += csz

            def emit_av(item):
                ih, qb, t0, csz, ex = item
                it = (ih, qb)
                b, hl = ih // HLOC, ih % HLOC
                for j in range(csz):
                    t = t0 + j
                    exsl = ex[:, j * QBLK : (j + 1) * QBLK]
                    nc.tensor.matmul(
                        av_pss[it][0:64, :],
                        lhsT=vn_sb[:, b, t, hl * 64 : hl * 64 + 64],
                        rhs=exsl,
                        start=(t == 0),
                        stop=(t == SKT - 1),
                    )
                    nc.tensor.matmul(
                        av_pss[it][64:65, :],
                        lhsT=km2_col(b, t),
                        rhs=exsl,
                        start=(t == 0),
                        stop=(t == SKT - 1),
                    )
                if t0 + csz == SKT:
                    drain_iter(it)

            pending = []  # depth-2 queue of chunks awaiting AV
            for ci, (ih, qb, t0, csz, tag) in enumerate(chunks):
                it = (ih, qb)
                if t0 == 0:
                    av_pss[it] = pacc.tile(
                        [65, QBLK], F32, tag="acc", name=f"av_ps{ih}_{qb}"
                    )
                pool = pA if tag == "A" else pB
                qk_ps = pool.tile(
                    [128, csz * QBLK], F32, tag="qk" + tag, name=f"qk_ps{ci}"
                )
                qsl = slice(qb * QBLK, (qb + 1) * QBLK)
                for j in range(csz):
                    t = t0 + j
                    nc.tensor.matmul(
                        qk_ps[:, j * QBLK : (j + 1) * QBLK],
                        lhsT=kt_sb[:, ih, t * 128 : (t + 1) * 128],
                        rhs=qt_sb[:, ih, qsl],
                        start=True,
                        stop=True,
                    )
                if len(pending) == 2:
                    emit_av(pending.pop(0))
                ex = epool.tile([128, csz * QBLK], FP16, tag="exp")
                nc.scalar.activation(
                    ex[:], qk_ps[:], mybir.ActivationFunctionType.Exp, scale=0.125
                )
                pending.append((ih, qb, t0, csz, ex))

            for item in pending:
                emit_av(item)

            # partial o-proj: y_part[jc, j, b, q] = sum_p wot[p, jc*128+j] outT[p, b, q]
            for jc in range(nj):
                for b in range(B):
                    for qb in range(nqb):
                        pool = pA if (jc * B * nqb + b * nqb + qb) % 2 == 0 else pB
                        y_ps = pool.tile(
                            [128, QBLK], F32, tag="qk" + ("A" if pool is pA else "B")
                        )
                        qsl = slice(qb * QBLK, (qb + 1) * QBLK)
                        nc.tensor.matmul(
                            y_ps[:],
                            lhsT=m16_sb[:, wot0 + jc * 128 : wot0 + (jc + 1) * 128],
                            rhs=outT_sb[:, b, qsl],
                            start=True,
                            stop=True,
                        )
                        y_sb = dpool.tile([128, QBLK], FP16, tag="y")
                        nc.vector.tensor_copy(y_sb[:], y_ps[:])
                        nc.sync.dma_start(y_part[jc, :, b, qsl], y_sb[:])

            # column-sharded o-proj all-reduce: each rank keeps jc == rank
            nc.gpsimd.collective_compute(
                "ReduceScatter",
                mybir.AluOpType.add,
                replica_groups=[list(range(NCORES))],
                ins=[y_part.opt()],
                outs=[y_rs.opt()],
            )

            # int8 downcast with per-feature-row scales: halves the download
            y_all = cpool.tile([128, B, SQ], FP16)
            nc.sync.dma_start(y_all[:], y_rs[:])
            mx_sb = cpool.tile([128, 1], F32)
            nc.vector.tensor_reduce(
                mx_sb[:],
                y_all[:],
                axis=mybir.AxisListType.XY,
                op=mybir.AluOpType.max,
                apply_absolute_value=True,
            )
            inv_sb = cpool.tile([128, 1], F32)
            nc.vector.tensor_scalar_add(inv_sb[:], mx_sb[:], 1e-30)
            nc.vector.reciprocal(inv_sb[:], inv_sb[:])
            nc.vector.tensor_scalar_mul(inv_sb[:], inv_sb[:], 127.0)
            yq_sb = cpool.tile([128, B, SQ], mybir.dt.int8)
            nc.scalar.activation(
                yq_sb[:], y_all[:], mybir.ActivationFunctionType.Copy, scale=inv_sb[:]
            )
            nc.sync.dma_start(yq[:], yq_sb[:])
            nc.sync.dma_start(mxo[:], mx_sb[:])

    nc.compile()
    return nc


def shard_inputs(query, key, value, key_mask, query_mask, Wo, bo):
    """Full inputs -> per-core input maps. Host does only casts + tiny prep;
    all layout transposes happen on device."""
    km01 = key_mask[:, :, 0] != 0  # [B, SK] bool
    qm01 = query_mask[:, :, 0] != 0  # [B, SQ]
    any_km = km01.any(axis=1)  # [B]

    qkv16 = np.empty((3, B, SQ, D), np.float16)
    qkv16[0] = query  # casts write straight into the blob
    qkv16[1] = key
    qkv16[2] = value

    km2 = np.ascontiguousarray(
        km01.astype(np.float16).reshape(B, SKT, 128).transpose(2, 0, 1)
    ).reshape(128, B * SKT)
    ident = np.eye(128, dtype=np.float16)
    woT = np.ascontiguousarray(Wo.T).astype(np.float16)  # [d, j]

    misc32 = np.empty((1, B * SQ + 64), np.float32)
    misc32[0, : B * SQ] = (qm01 & any_km[:, None]).reshape(B * SQ)
    misc32[0, B * SQ :] = 1.0

    in_maps = []
    for c in range(NCORES):
        dsl = slice(128 * c, 128 * (c + 1))  # heads 2c, 2c+1
        misc16 = np.concatenate([km2, ident, woT[dsl]], axis=1)
        in_maps.append(
            {
                "qkv": qkv16[:, :, :, dsl],
                "misc16": misc16,
                "misc32": misc32,
            }
        )
    return in_maps


_NC_CACHE = {}


def _get_program():
    if "nc" not in _NC_CACHE:
        _NC_CACHE["nc"] = build_program()
    return _NC_CACHE["nc"]


def kernel(query, key, value, key_mask, query_mask, Wo, bo, _trace=False):
    query = np.asarray(query, dtype=np.float32)
    key = np.asarray(key, dtype=np.float32)
    value = np.asarray(value, dtype=np.float32)
    key_mask = np.asarray(key_mask, dtype=np.int32)
    query_mask = np.asarray(query_mask, dtype=np.int32)
    Wo = np.asarray(Wo, dtype=np.float32)
    bo = np.asarray(bo, dtype=np.float32)

    nc = _get_program()
    in_maps = shard_inputs(query, key, value, key_mask, query_mask, Wo, bo)
    try:
        res = bass_utils.run_bass_kernel_spmd(
            nc, in_maps, core_ids=list(range(NCORES)), trace=_trace
        )
    except ModuleNotFoundError:
        # axon NTFF profile hook unavailable in this container; run untraced
        res = bass_utils.run_bass_kernel_spmd(
            nc, in_maps, core_ids=list(range(NCORES)), trace=False
        )
    kernel.last_results = res

    out = np.empty((B, SQ, D), np.float32)
    for c in range(NCORES):
        ytq = res.results[c]["yq"]  # int8 [128, B, SQ], features 128c..128c+127
        sc = res.results[c]["mx"] * np.float32(1.0 / 127.0)  # [128, 1]
        jsl = slice(128 * c, 128 * (c + 1))
        for g in range(B):
            out[g, :, jsl] = (ytq[:, g, :] * sc).T + bo[jsl]
    return out


# revision 22
# speedup vs baseline: 1.2401x; 1.2023x over previous
"""Trainium2 Bass kernel for nn_CrossAttention (b=2, sq=sk=2048, d=1024, h=16).

Wire-optimized sharding: per-call wall clock is dominated by host<->device
transfer over the axon tunnel (~100MB/s, serialized across cores), so every
tensor is shipped exactly once in fp16 with no cross-core replication:
each of the 8 cores owns 2 heads x both batches (a contiguous 128-column
d-slice of q/k/v in natural [s, d] layout — host does only fp16 casts, all
layout work happens on device). The o_proj weight is d-sharded 8-way and the
partial yT[j, b, q] outputs are summed with an on-device ReduceScatter; each
core downloads its disjoint 128-feature slice quantized to int8 with
per-feature-row scales (rel err ~0.4% vs the 2e-2 budget).

Per-core device pipeline:
  qT/kT tiles  = PE transpose of natural q/k tiles      (PE + DVE copy)
  vn          *= km (key mask, multiplicative)          (DVE)
  scoresT[s,q] = sum_d K[s,d] Q[q,d]                    (PE, fp16 -> f32 psum)
  expT         = exp(0.125 * scoresT)                   (ACT, psum->sbuf fp16)
  avT[0:64,q]  = sum_s vn[s,m] expT[s,q]                (PE, accumulated)
  avT[64,q]    = sum_s km[s] expT[s,q]   (denominator)  (PE, accumulated)
  sc[q]        = rs[q] / (avT[64,q] + eps)              (DVE on 1 partition)
  bc[i,q]      = sc[q]                                  (PE K=1 broadcast)
  outT[...]    = avT[i,q] * bc[i,q]                     (DVE -> fp16)
  y_part       = WoT-slice @ outT                       (PE)
  y_rs         = ReduceScatter_add(y_part, ranks 0..7)  (rank keeps jc==rank)
  yq, mx       = int8 quantize with per-row absmax      (DVE + ACT)

Query-mask / fully-masked rows are zeroed by rs, matching the reference's
nan_to_num semantics. Host adds bo and dequantizes during assembly.
"""

import numpy as np

import concourse.mybir as mybir
import concourse.tile as tile
from concourse import bacc
from concourse import bass_utils

FP16 = mybir.dt.float16
F32 = mybir.dt.float32

# full-problem constants
B, SQ, SK, D, H, HD = 2, 2048, 2048, 1024, 16, 64
NCORES = 8
HLOC = H // NCORES  # 2 heads per core
QBLK = 512  # q columns per psum tile
SKT = SK // 128  # 16 s tiles (same count for q and k)

# per-iteration sk-tile chunking: sizes sum to SKT, tags strictly alternate
# so psum-slot reuse distance stays >= 2 across iteration boundaries
CHUNK_PLAN = [(3, "A"), (3, "B"), (3, "A"), (3, "B"), (2, "A"), (2, "B")]


def build_program():
    nih = B * HLOC  # 4 (batch, local-head) pairs
    nqb = SQ // QBLK  # 4 q blocks
    nj = D // 128  # 8 output-feature chunks (ReduceScatter dim)
    nc = bacc.Bacc(
        "TRN2",
        target_bir_lowering=False,
        debug=False,
        enable_asserts=False,
        num_devices=NCORES,
    )

    # inputs merged into 3 arrays (per-array transfer overhead is ~7ms);
    # q/k/v ship as int8 with per-row scales (decoded to fp16 on device)
    qkv = nc.dram_tensor("qkv", [3, B, SQ, 128], mybir.dt.int8, kind="ExternalInput").ap()
    # misc16 cols: scales [0:3*B*SKT] | km2 [.. +B*SKT] | ident [.. +128] | wot [.. +D]
    NSC = 3 * B * SKT  # 96 scale columns
    misc16 = nc.dram_tensor(
        "misc16", [128, NSC + B * SKT + 128 + D], FP16, kind="ExternalInput"
    ).ap()
    # misc32 cols: rs [0:B*SQ] | ones [B*SQ:B*SQ+64]
    misc32 = nc.dram_tensor("misc32", [1, B * SQ + 64], F32, kind="ExternalInput").ap()
    yq = nc.dram_tensor("yq", [128, B, SQ], mybir.dt.int8, kind="ExternalOutput").ap()
    mxo = nc.dram_tensor("mx", [128, 1], F32, kind="ExternalOutput").ap()

    with tile.TileContext(nc) as tc:
        with (
            tc.tile_pool(name="const", bufs=1) as cpool,
            tc.tile_pool(name="exp", bufs=4) as epool,
            tc.tile_pool(name="drain", bufs=2) as dpool,
            tc.tile_pool(name="pA", bufs=1, space="PSUM") as pA,
            tc.tile_pool(name="pB", bufs=1, space="PSUM") as pB,
            tc.tile_pool(name="pacc", bufs=1, space="PSUM") as pacc,
            tc.tile_pool(name="pbc", bufs=1, space="PSUM") as pbc,
            tc.tile_pool(name="dram", bufs=1, space="DRAM") as dram,
        ):
            q8_sb = cpool.tile([128, B, SKT, 128], mybir.dt.int8)
            k8_sb = cpool.tile([128, B, SKT, 128], mybir.dt.int8)
            v8_sb = cpool.tile([128, B, SKT, 128], mybir.dt.int8)
            qn_sb = cpool.tile([128, B, SKT, 128], FP16)
            kn_sb = cpool.tile([128, B, SKT, 128], FP16)
            vn_sb = cpool.tile([128, B, SKT, 128], FP16)
            m16_sb = cpool.tile([128, NSC + B * SKT + 128 + D], FP16)
            m32_sb = cpool.tile([1, B * SQ + 64], F32)
            qt_sb = cpool.tile([64, nih, SQ], FP16)
            kt_sb = cpool.tile([64, nih, SK], FP16)
            outT_sb = cpool.tile([128, B, SQ], FP16)

            def sc_col(i, b, t):  # [128, 1] decode-scale column (tensor i)
                j = (i * B + b) * SKT + t
                return m16_sb[:, j : j + 1]

            def km2_col(b, t):  # [128, 1] key-mask column for (batch, sk-tile)
                j = NSC + b * SKT + t
                return m16_sb[:, j : j + 1]

            ident_sb = m16_sb[:, NSC + B * SKT : NSC + B * SKT + 128]
            wot0 = NSC + B * SKT + 128  # wot column base inside m16_sb
            rs_sb = m32_sb[:, 0 : B * SQ]
            ones_sb = m32_sb[:, B * SQ : B * SQ + 64]

            y_part = dram.tile([nj, 128, B, SQ], FP16)
            y_rs = dram.tile([128, B, SQ], FP16)

            for b in range(B):
                nc.sync.dma_start(
                    q8_sb[:, b], qkv[0, b].rearrange("(t p) d -> p t d", p=128)
                )
                nc.sync.dma_start(
                    k8_sb[:, b], qkv[1, b].rearrange("(t p) d -> p t d", p=128)
                )
                nc.sync.dma_start(
                    v8_sb[:, b], qkv[2, b].rearrange("(t p) d -> p t d", p=128)
                )
            nc.sync.dma_start(m16_sb[:], misc16[:])
            nc.sync.dma_start(m32_sb[:], misc32[:])

            # decode int8 -> fp16 with per-row scales (v's scale has the key
            # mask pre-folded on host, so no separate mask multiply is needed)
            for i, (src, dst) in enumerate(
                ((q8_sb, qn_sb), (k8_sb, kn_sb), (v8_sb, vn_sb))
            ):
                for b in range(B):
                    for t in range(SKT):
                        nc.vector.tensor_tensor(
                            dst[:, b, t, :],
                            src[:, b, t, :],
                            sc_col(i, b, t).to_broadcast((128, 128)),
                            mybir.AluOpType.mult,
                        )

            # PE-transpose natural q/k tiles into [hd, s] operand layout
            tpools = (pacc, pbc)
            ttags = ("acc", "bc")
            idx = 0
            for ih in range(nih):
                b, hl = ih // HLOC, ih % HLOC
                for t in range(SKT):
                    for src, dst in ((qn_sb, qt_sb), (kn_sb, kt_sb)):
                        tp = tpools[idx % 2].tile([64, 128], FP16, tag=ttags[idx % 2])
                        idx += 1
                        nc.tensor.transpose(
                            tp[:], src[:, b, t, hl * 64 : hl * 64 + 64], ident_sb[:]
                        )
                        nc.vector.tensor_copy(dst[:, ih, t * 128 : (t + 1) * 128], tp[:])

            av_pss = {}

            def drain_iter(it):
                ih, qb = it
                b = ih // HLOC
                hl = ih % HLOC
                qsl = slice(qb * QBLK, (qb + 1) * QBLK)
                av_sb = dpool.tile([65, QBLK], F32, tag="avsb")
                nc.vector.tensor_copy(av_sb[:], av_pss[it][:])
                sc = dpool.tile([1, QBLK], F32, tag="sc")
                nc.vector.tensor_scalar_add(sc[:], av_sb[64:65, :], 1e-30)
                nc.vector.reciprocal(sc[:], sc[:])
                nc.vector.tensor_mul(
                    sc[:],
                    sc[:],
                    rs_sb[0:1, b * SQ + qb * QBLK : b * SQ + (qb + 1) * QBLK],
                )
                bc_ps = pbc.tile([64, QBLK], F32, tag="bc")
                nc.tensor.matmul(
                    bc_ps[:], lhsT=ones_sb[:], rhs=sc[:], start=True, stop=True
                )
                nc.vector.tensor_tensor(
                    outT_sb[hl * 64 : hl * 64 + 64, b, qsl],
                    av_sb[0:64, :],
                    bc_ps[:],
                    mybir.AluOpType.mult,
                )

            # flat, software-pipelined chunk stream: QK(c+1) is emitted
            # before AV(c) so the in-order PE queue never waits on exp(c)
            chunks = []
            for ih in range(nih):
                for qb in range(nqb):
                    t0 = 0
                    for csz, tag in CHUNK_PLAN:
                        chunks.append((ih, qb, t0, csz, tag))
                        t0 += csz

            def emit_av(item):
                ih, qb, t0, csz, ex = item
                it = (ih, qb)
                b, hl = ih // HLOC, ih % HLOC
                for j in range(csz):
                    t = t0 + j
                    exsl = ex[:, j * QBLK : (j + 1) * QBLK]
                    nc.tensor.matmul(
                        av_pss[it][0:64, :],
                        lhsT=vn_sb[:, b, t, hl * 64 : hl * 64 + 64],
                        rhs=exsl,
                        start=(t == 0),
                        stop=(t == SKT - 1),
                    )
                    nc.tensor.matmul(
                        av_pss[it][64:65, :],
                        lhsT=km2_col(b, t),
                        rhs=exsl,
                        start=(t == 0),
                        stop=(t == SKT - 1),
                    )
                if t0 + csz == SKT:
                    drain_iter(it)

            pending = []  # depth-2 queue of chunks awaiting AV
            for ci, (ih, qb, t0, csz, tag) in enumerate(chunks):
                it = (ih, qb)
                if t0 == 0:
                    av_pss[it] = pacc.tile(
                        [65, QBLK], F32, tag="acc", name=f"av_ps{ih}_{qb}"
                    )
                pool = pA if tag == "A" else pB
                qk_ps = pool.tile(
                    [128, csz * QBLK], F32, tag="qk" + tag, name=f"qk_ps{ci}"
                )
                qsl = slice(qb * QBLK, (qb + 1) * QBLK)
                for j in range(csz):
                    t = t0 + j
                    nc.tensor.matmul(
                        qk_ps[:, j * QBLK : (j + 1) * QBLK],
                        lhsT=kt_sb[:, ih, t * 128 : (t + 1) * 128],
                        rhs=qt_sb[:, ih, qsl],
                        start=True,
                        stop=True,
                    )
                if len(pending) == 2:
                    emit_av(pending.pop(0))
                ex = epool.tile([128, csz * QBLK], FP16, tag="exp")
                nc.scalar.activation(
                    ex[:], qk_ps[:], mybir.ActivationFunctionType.Exp, scale=0.125
                )
                pending.append((ih, qb, t0, csz, ex))

            for item in pending:
                emit_av(item)

            # partial o-proj: y_part[jc, j, b, q] = sum_p wot[p, jc*128+j] outT[p, b, q]
            for jc in range(nj):
                for b in range(B):
                    for qb in range(nqb):
                        pool = pA if (jc * B * nqb + b * nqb + qb) % 2 == 0 else pB
                        y_ps = pool.tile(
                            [128, QBLK], F32, tag="qk" + ("A" if pool is pA else "B")
                        )
                        qsl = slice(qb * QBLK, (qb + 1) * QBLK)
                        nc.tensor.matmul(
                            y_ps[:],
                            lhsT=m16_sb[:, wot0 + jc * 128 : wot0 + (jc + 1) * 128],
                            rhs=outT_sb[:, b, qsl],
                            start=True,
                            stop=True,
                        )
                        y_sb = dpool.tile([128, QBLK], FP16, tag="y")
                        nc.vector.tensor_copy(y_sb[:], y_ps[:])
                        nc.sync.dma_start(y_part[jc, :, b, qsl], y_sb[:])

            # column-sharded o-proj all-reduce: each rank keeps jc == rank
            nc.gpsimd.collective_compute(
                "ReduceScatter",
                mybir.AluOpType.add,
                replica_groups=[list(range(NCORES))],
                ins=[y_part.opt()],
                outs=[y_rs.opt()],
            )

            # int8 downcast with per-feature-row scales: halves the download
            y_all = cpool.tile([128, B, SQ], FP16)
            nc.sync.dma_start(y_all[:], y_rs[:])
            mx_sb = cpool.tile([128, 1], F32)
            nc.vector.tensor_reduce(
                mx_sb[:],
                y_all[:],
                axis=mybir.AxisListType.XY,
                op=mybir.AluOpType.max,
                apply_absolute_value=True,
            )
            inv_sb = cpool.tile([128, 1], F32)
            nc.vector.tensor_scalar_add(inv_sb[:], mx_sb[:], 1e-30)
            nc.vector.reciprocal(inv_sb[:], inv_sb[:])
            nc.vector.tensor_scalar_mul(inv_sb[:], inv_sb[:], 127.0)
            yq_sb = cpool.tile([128, B, SQ], mybir.dt.int8)
            nc.scalar.activation(
                yq_sb[:], y_all[:], mybir.ActivationFunctionType.Copy, scale=inv_sb[:]
            )
            nc.sync.dma_start(yq[:], yq_sb[:])
            nc.sync.dma_start(mxo[:], mx_sb[:])

    nc.compile()
    return nc


def shard_inputs(query, key, value, key_mask, query_mask, Wo, bo):
    """Full inputs -> per-core input maps. Host does only casts + tiny prep;
    all layout transposes happen on device."""
    km01 = key_mask[:, :, 0] != 0  # [B, SK] bool
    qm01 = query_mask[:, :, 0] != 0  # [B, SQ]
    any_km = km01.any(axis=1)  # [B]

    # int8 quantize q/k/v with per-(row, 128-col core slice) scales; the key
    # mask is pre-folded into v's decode scale so masked rows decode to 0
    qkv8 = np.empty((3, B, SQ, NCORES, 128), np.int8)
    scales = np.empty((3, B, SQ, NCORES), np.float32)
    tmp = np.empty((B, SQ, NCORES, 128), np.float32)
    for i, x in enumerate((query, key, value)):
        xb = x.reshape(B, SQ, NCORES, 128)
        mx = np.abs(xb).max(axis=-1)  # [B, SQ, NCORES]
        inv = np.float32(127.0) / (mx + np.float32(1e-30))
        np.multiply(xb, inv[..., None], out=tmp)
        np.rint(tmp, out=tmp)
        qkv8[i] = tmp
        scales[i] = mx * np.float32(1.0 / 127.0)
    scales[2] *= km01[:, :, None]  # fold key mask into v decode
    # per-core scale columns: [128, 3, B, SKT] with partition = s % 128
    sc_all = np.ascontiguousarray(
        scales.astype(np.float16)
        .reshape(3, B, SKT, 128, NCORES)
        .transpose(4, 3, 0, 1, 2)
    )  # [NCORES, 128, 3, B, SKT]

    km2 = np.ascontiguousarray(
        km01.astype(np.float16).reshape(B, SKT, 128).transpose(2, 0, 1)
    ).reshape(128, B * SKT)
    ident = np.eye(128, dtype=np.float16)
    woT = np.ascontiguousarray(Wo.T).astype(np.float16)  # [d, j]

    misc32 = np.empty((1, B * SQ + 64), np.float32)
    misc32[0, : B * SQ] = (qm01 & any_km[:, None]).reshape(B * SQ)
    misc32[0, B * SQ :] = 1.0

    in_maps = []
    for c in range(NCORES):
        dsl = slice(128 * c, 128 * (c + 1))  # heads 2c, 2c+1
        misc16 = np.concatenate(
            [sc_all[c].reshape(128, 3 * B * SKT), km2, ident, woT[dsl]], axis=1
        )
        in_maps.append(
            {
                "qkv": qkv8[:, :, :, c, :],
                "misc16": misc16,
                "misc32": misc32,
            }
        )
    return in_maps


_NC_CACHE = {}


def _get_program():
    if "nc" not in _NC_CACHE:
        _NC_CACHE["nc"] = build_program()
    return _NC_CACHE["nc"]


def kernel(query, key, value, key_mask, query_mask, Wo, bo, _trace=False):
    query = np.asarray(query, dtype=np.float32)
    key = np.asarray(key, dtype=np.float32)
    value = np.asarray(value, dtype=np.float32)
    key_mask = np.asarray(key_mask, dtype=np.int32)
    query_mask = np.asarray(query_mask, dtype=np.int32)
    Wo = np.asarray(Wo, dtype=np.float32)
    bo = np.asarray(bo, dtype=np.float32)

    nc = _get_program()
    in_maps = shard_inputs(query, key, value, key_mask, query_mask, Wo, bo)
    try:
        res = bass_utils.run_bass_kernel_spmd(
            nc, in_maps, core_ids=list(range(NCORES)), trace=_trace
        )
    except ModuleNotFoundError:
        # axon NTFF profile hook unavailable in this container; run untraced
        res = bass_utils.run_bass_kernel_spmd(
            nc, in_maps, core_ids=list(range(NCORES)), trace=False
        )
    kernel.last_results = res

    out = np.empty((B, SQ, D), np.float32)
    for c in range(NCORES):
        ytq = res.results[c]["yq"]  # int8 [128, B, SQ], features 128c..128c+127
        sc = res.results[c]["mx"] * np.float32(1.0 / 127.0)  # [128, 1]
        jsl = slice(128 * c, 128 * (c + 1))
        for g in range(B):
            out[g, :, jsl] = (ytq[:, g, :] * sc).T + bo[jsl]
    return out


# revision 27
# speedup vs baseline: 1.3564x; 1.0938x over previous
"""Trainium2 Bass kernel for nn_CrossAttention (b=2, sq=sk=2048, d=1024, h=16).

Wire-optimized sharding: per-call wall clock is dominated by host<->device
transfer over the axon tunnel (~100MB/s, serialized across cores), so every
tensor is shipped exactly once in fp16 with no cross-core replication:
each of the 8 cores owns 2 heads x both batches (a contiguous 128-column
d-slice of q/k/v in natural [s, d] layout — host does only fp16 casts, all
layout work happens on device). The o_proj weight is d-sharded 8-way and the
partial yT[j, b, q] outputs are summed with an on-device ReduceScatter; each
core downloads its disjoint 128-feature slice quantized to int8 with
per-feature-row scales (rel err ~0.4% vs the 2e-2 budget).

Per-core device pipeline:
  qT/kT tiles  = PE transpose of natural q/k tiles      (PE + DVE copy)
  vn          *= km (key mask, multiplicative)          (DVE)
  scoresT[s,q] = sum_d K[s,d] Q[q,d]                    (PE, fp16 -> f32 psum)
  expT         = exp(0.125 * scoresT)                   (ACT, psum->sbuf fp16)
  avT[0:64,q]  = sum_s vn[s,m] expT[s,q]                (PE, accumulated)
  avT[64,q]    = sum_s km[s] expT[s,q]   (denominator)  (PE, accumulated)
  sc[q]        = rs[q] / (avT[64,q] + eps)              (DVE on 1 partition)
  bc[i,q]      = sc[q]                                  (PE K=1 broadcast)
  outT[...]    = avT[i,q] * bc[i,q]                     (DVE -> fp16)
  y_part       = WoT-slice @ outT                       (PE)
  y_rs         = ReduceScatter_add(y_part, ranks 0..7)  (rank keeps jc==rank)
  yq, mx       = int8 quantize with per-row absmax      (DVE + ACT)

Query-mask / fully-masked rows are zeroed by rs, matching the reference's
nan_to_num semantics. Host adds bo and dequantizes during assembly.
"""

import numpy as np

import concourse.mybir as mybir
import concourse.tile as tile
from concourse import bacc
from concourse import bass_utils

FP16 = mybir.dt.float16
F32 = mybir.dt.float32

# full-problem constants
B, SQ, SK, D, H, HD = 2, 2048, 2048, 1024, 16, 64
NCORES = 8
HLOC = H // NCORES  # 2 heads per core
QBLK = 512  # q columns per psum tile
SKT = SK // 128  # 16 s tiles (same count for q and k)

# per-iteration sk-tile chunking: sizes sum to SKT, tags strictly alternate
# so psum-slot reuse distance stays >= 2 across iteration boundaries
CHUNK_PLAN = [(3, "A"), (3, "B"), (3, "A"), (3, "B"), (2, "A"), (2, "B")]


def build_program():
    nih = B * HLOC  # 4 (batch, local-head) pairs
    nqb = SQ // QBLK  # 4 q blocks
    nj = D // 128  # 8 output-feature chunks (ReduceScatter dim)
    nc = bacc.Bacc(
        "TRN2",
        target_bir_lowering=False,
        debug=False,
        enable_asserts=False,
        num_devices=NCORES,
    )

    # all inputs ship as ONE int8 blob per core (each extra array costs ~7ms
    # of transfer dispatch): qkv int8 | misc16 (fp16, bitcast) | rs (fp16)
    NSC = 3 * B * SKT  # 96 decode-scale columns
    MCOLS = NSC + B * SKT + 128 + D  # scales | km2 | ident | wot
    QOFF = 0
    QBYTES = 3 * B * SQ * 128
    MOFF = QOFF + QBYTES
    MBYTES = 128 * MCOLS * 2
    ROFF = MOFF + MBYTES
    RBYTES = B * SQ * 2
    NBLOB = ROFF + RBYTES
    blob = nc.dram_tensor("blob", [NBLOB], mybir.dt.int8, kind="ExternalInput").ap()
    yq = nc.dram_tensor("yq", [128, B, SQ], mybir.dt.int8, kind="ExternalOutput").ap()
    mxo = nc.dram_tensor("mx", [128, 1], F32, kind="ExternalOutput").ap()

    with tile.TileContext(nc) as tc:
        with (
            tc.tile_pool(name="const", bufs=1) as cpool,
            tc.tile_pool(name="exp", bufs=4) as epool,
            tc.tile_pool(name="drain", bufs=2) as dpool,
            tc.tile_pool(name="pA", bufs=1, space="PSUM") as pA,
            tc.tile_pool(name="pB", bufs=1, space="PSUM") as pB,
            tc.tile_pool(name="pacc", bufs=1, space="PSUM") as pacc,
            tc.tile_pool(name="pbc", bufs=1, space="PSUM") as pbc,
            tc.tile_pool(name="dram", bufs=1, space="DRAM") as dram,
        ):
            q8_sb = cpool.tile([128, B, SKT, 128], mybir.dt.int8)
            k8_sb = cpool.tile([128, B, SKT, 128], mybir.dt.int8)
            v8_sb = cpool.tile([128, B, SKT, 128], mybir.dt.int8)
            qn_sb = cpool.tile([128, B, SKT, 128], FP16)
            kn_sb = cpool.tile([128, B, SKT, 128], FP16)
            vn_sb = cpool.tile([128, B, SKT, 128], FP16)
            m16_sb = cpool.tile([128, MCOLS], FP16)
            rs_sb = cpool.tile([1, B * SQ], FP16)
            ones_sb = cpool.tile([1, 64], F32)
            qt_sb = cpool.tile([64, nih, SQ], FP16)
            kt_sb = cpool.tile([64, nih, SK], FP16)
            outT_sb = cpool.tile([128, B, SQ], FP16)

            def sc_col(i, b, t):  # [128, 1] decode-scale column (tensor i)
                j = (i * B + b) * SKT + t
                return m16_sb[:, j : j + 1]

            def km2_col(b, t):  # [128, 1] key-mask column for (batch, sk-tile)
                j = NSC + b * SKT + t
                return m16_sb[:, j : j + 1]

            ident_sb = m16_sb[:, NSC + B * SKT : NSC + B * SKT + 128]
            wot0 = NSC + B * SKT + 128  # wot column base inside m16_sb

            y_part = dram.tile([nj, 128, B, SQ], FP16)
            y_rs = dram.tile([128, B, SQ], FP16)

            for i, dst8 in enumerate((q8_sb, k8_sb, v8_sb)):
                for b in range(B):
                    off = (i * B + b) * SQ * 128
                    nc.sync.dma_start(
                        dst8[:, b],
                        blob[off : off + SQ * 128].rearrange(
                            "(t p d) -> p t d", p=128, d=128
                        ),
                    )
            nc.sync.dma_start(
                m16_sb[:],
                blob[MOFF : MOFF + MBYTES].bitcast(FP16).rearrange(
                    "(p c) -> p c", p=128
                ),
            )
            nc.sync.dma_start(
                rs_sb[:],
                blob[ROFF : ROFF + RBYTES].bitcast(FP16).rearrange(
                    "(o c) -> o c", o=1
                ),
            )
            # ones vector for the denominator-broadcast matmul, built on device
            nc.vector.memzero(ones_sb[:])
            nc.vector.tensor_scalar_add(ones_sb[:], ones_sb[:], 1.0)

            # decode int8 -> fp16 with per-row scales (v's scale has the key
            # mask pre-folded on host, so no separate mask multiply is needed)
            for i, (src, dst) in enumerate(
                ((q8_sb, qn_sb), (k8_sb, kn_sb), (v8_sb, vn_sb))
            ):
                for b in range(B):
                    for t in range(SKT):
                        nc.vector.tensor_tensor(
                            dst[:, b, t, :],
                            src[:, b, t, :],
                            sc_col(i, b, t).to_broadcast((128, 128)),
                            mybir.AluOpType.mult,
                        )

            # PE-transpose natural q/k tiles into [hd, s] operand layout
            tpools = (pacc, pbc)
            ttags = ("acc", "bc")
            idx = 0
            for ih in range(nih):
                b, hl = ih // HLOC, ih % HLOC
                for t in range(SKT):
                    for src, dst in ((qn_sb, qt_sb), (kn_sb, kt_sb)):
                        tp = tpools[idx % 2].tile([64, 128], FP16, tag=ttags[idx % 2])
                        idx += 1
                        nc.tensor.transpose(
                            tp[:], src[:, b, t, hl * 64 : hl * 64 + 64], ident_sb[:]
                        )
                        nc.vector.tensor_copy(dst[:, ih, t * 128 : (t + 1) * 128], tp[:])

            av_pss = {}

            def drain_iter(it):
                ih, qb = it
                b = ih // HLOC
                hl = ih % HLOC
                qsl = slice(qb * QBLK, (qb + 1) * QBLK)
                av_sb = dpool.tile([65, QBLK], F32, tag="avsb")
                nc.vector.tensor_copy(av_sb[:], av_pss[it][:])
                sc = dpool.tile([1, QBLK], F32, tag="sc")
                nc.vector.tensor_scalar_add(sc[:], av_sb[64:65, :], 1e-30)
                nc.vector.reciprocal(sc[:], sc[:])
                nc.vector.tensor_mul(
                    sc[:],
                    sc[:],
                    rs_sb[0:1, b * SQ + qb * QBLK : b * SQ + (qb + 1) * QBLK],
                )
                bc_ps = pbc.tile([64, QBLK], F32, tag="bc")
                nc.tensor.matmul(
                    bc_ps[:], lhsT=ones_sb[:], rhs=sc[:], start=True, stop=True
                )
                nc.vector.tensor_tensor(
                    outT_sb[hl * 64 : hl * 64 + 64, b, qsl],
                    av_sb[0:64, :],
                    bc_ps[:],
                    mybir.AluOpType.mult,
                )

            # flat, software-pipelined chunk stream: QK(c+1) is emitted
            # before AV(c) so the in-order PE queue never waits on exp(c)
            chunks = []
            for ih in range(nih):
                for qb in range(nqb):
                    t0 = 0
                    for csz, tag in CHUNK_PLAN:
                        chunks.append((ih, qb, t0, csz, tag))
                        t0 += csz

            def emit_av(item):
                ih, qb, t0, csz, ex = item
                it = (ih, qb)
                b, hl = ih // HLOC, ih % HLOC
                for j in range(csz):
                    t = t0 + j
                    exsl = ex[:, j * QBLK : (j + 1) * QBLK]
                    nc.tensor.matmul(
                        av_pss[it][0:64, :],
                        lhsT=vn_sb[:, b, t, hl * 64 : hl * 64 + 64],
                        rhs=exsl,
                        start=(t == 0),
                        stop=(t == SKT - 1),
                    )
                    nc.tensor.matmul(
                        av_pss[it][64:65, :],
                        lhsT=km2_col(b, t),
                        rhs=exsl,
                        start=(t == 0),
                        stop=(t == SKT - 1),
                    )
                if t0 + csz == SKT:
                    drain_iter(it)

            pending = []  # depth-2 queue of chunks awaiting AV
            for ci, (ih, qb, t0, csz, tag) in enumerate(chunks):
                it = (ih, qb)
                if t0 == 0:
                    av_pss[it] = pacc.tile(
                        [65, QBLK], F32, tag="acc", name=f"av_ps{ih}_{qb}"
                    )
                pool = pA if tag == "A" else pB
                qk_ps = pool.tile(
                    [128, csz * QBLK], F32, tag="qk" + tag, name=f"qk_ps{ci}"
                )
                qsl = slice(qb * QBLK, (qb + 1) * QBLK)
                for j in range(csz):
                    t = t0 + j
                    nc.tensor.matmul(
                        qk_ps[:, j * QBLK : (j + 1) * QBLK],
                        lhsT=kt_sb[:, ih, t * 128 : (t + 1) * 128],
                        rhs=qt_sb[:, ih, qsl],
                        start=True,
                        stop=True,
                    )
                if len(pending) == 2:
                    emit_av(pending.pop(0))
                ex = epool.tile([128, csz * QBLK], FP16, tag="exp")
                nc.scalar.activation(
                    ex[:], qk_ps[:], mybir.ActivationFunctionType.Exp, scale=0.125
                )
                pending.append((ih, qb, t0, csz, ex))

            for item in pending:
                emit_av(item)

            # partial o-proj: y_part[jc, j, b, q] = sum_p wot[p, jc*128+j] outT[p, b, q]
            for jc in range(nj):
                for b in range(B):
                    for qb in range(nqb):
                        pool = pA if (jc * B * nqb + b * nqb + qb) % 2 == 0 else pB
                        y_ps = pool.tile(
                            [128, QBLK], F32, tag="qk" + ("A" if pool is pA else "B")
                        )
                        qsl = slice(qb * QBLK, (qb + 1) * QBLK)
                        nc.tensor.matmul(
                            y_ps[:],
                            lhsT=m16_sb[:, wot0 + jc * 128 : wot0 + (jc + 1) * 128],
                            rhs=outT_sb[:, b, qsl],
                            start=True,
                            stop=True,
                        )
                        y_sb = dpool.tile([128, QBLK], FP16, tag="y")
                        nc.vector.tensor_copy(y_sb[:], y_ps[:])
                        nc.sync.dma_start(y_part[jc, :, b, qsl], y_sb[:])

            # column-sharded o-proj all-reduce: each rank keeps jc == rank
            nc.gpsimd.collective_compute(
                "ReduceScatter",
                mybir.AluOpType.add,
                replica_groups=[list(range(NCORES))],
                ins=[y_part.opt()],
                outs=[y_rs.opt()],
            )

            # int8 downcast with per-feature-row scales: halves the download
            y_all = cpool.tile([128, B, SQ], FP16)
            nc.sync.dma_start(y_all[:], y_rs[:])
            mx_sb = cpool.tile([128, 1], F32)
            nc.vector.tensor_reduce(
                mx_sb[:],
                y_all[:],
                axis=mybir.AxisListType.XY,
                op=mybir.AluOpType.max,
                apply_absolute_value=True,
            )
            inv_sb = cpool.tile([128, 1], F32)
            nc.vector.tensor_scalar_add(inv_sb[:], mx_sb[:], 1e-30)
            nc.vector.reciprocal(inv_sb[:], inv_sb[:])
            nc.vector.tensor_scalar_mul(inv_sb[:], inv_sb[:], 127.0)
            yq_sb = cpool.tile([128, B, SQ], mybir.dt.int8)
            nc.scalar.activation(
                yq_sb[:], y_all[:], mybir.ActivationFunctionType.Copy, scale=inv_sb[:]
            )
            nc.sync.dma_start(yq[:], yq_sb[:])
            nc.sync.dma_start(mxo[:], mx_sb[:])

    nc.compile()
    return nc


def shard_inputs(query, key, value, key_mask, query_mask, Wo, bo):
    """Full inputs -> per-core input maps. Host does only casts + tiny prep;
    all layout transposes happen on device."""
    km01 = key_mask[:, :, 0] != 0  # [B, SK] bool
    qm01 = query_mask[:, :, 0] != 0  # [B, SQ]
    any_km = km01.any(axis=1)  # [B]

    # blob layout must match build_program: qkv int8 | misc16 fp16 | rs fp16
    QBYTES = 3 * B * SQ * 128
    MCOLS = 3 * B * SKT + B * SKT + 128 + D  # scales | km2 | ident | wot
    MBYTES = 128 * MCOLS * 2
    ROFF = QBYTES + MBYTES
    NBLOB = ROFF + B * SQ * 2
    blob = np.empty((NCORES, NBLOB), np.int8)

    # int8 quantize q/k/v with per-(row, 128-col core slice) scales; the key
    # mask is pre-folded into v's decode scale so masked rows decode to 0.
    # qdst aliases the blob so the quantized bytes land per-core directly.
    qdst = blob[:, :QBYTES].reshape(NCORES, 3, B, SQ, 128).transpose(1, 2, 3, 0, 4)
    scales = np.empty((3, B, SQ, NCORES), np.float32)
    tmp = np.empty((B, SQ, NCORES, 128), np.float32)
    for i, x in enumerate((query, key, value)):
        xb = x.reshape(B, SQ, NCORES, 128)
        mx = np.abs(xb).max(axis=-1)  # [B, SQ, NCORES]
        inv = np.float32(127.0) / (mx + np.float32(1e-30))
        np.multiply(xb, inv[..., None], out=tmp)
        np.rint(tmp, out=tmp)
        qdst[i] = tmp
        scales[i] = mx * np.float32(1.0 / 127.0)
    scales[2] *= km01[:, :, None]  # fold key mask into v decode
    # per-core scale columns: [128, 3, B, SKT] with partition = s % 128
    sc_all = np.ascontiguousarray(
        scales.astype(np.float16)
        .reshape(3, B, SKT, 128, NCORES)
        .transpose(4, 3, 0, 1, 2)
    )  # [NCORES, 128, 3, B, SKT]

    km2 = np.ascontiguousarray(
        km01.astype(np.float16).reshape(B, SKT, 128).transpose(2, 0, 1)
    ).reshape(128, B * SKT)
    ident = np.eye(128, dtype=np.float16)
    woT = np.ascontiguousarray(Wo.T).astype(np.float16)  # [d, j]
    rs16 = (qm01 & any_km[:, None]).astype(np.float16).reshape(B * SQ)

    NSC = 3 * B * SKT
    for c in range(NCORES):
        m16 = blob[c, QBYTES:ROFF].view(np.float16).reshape(128, MCOLS)
        m16[:, :NSC] = sc_all[c].reshape(128, NSC)
        m16[:, NSC : NSC + B * SKT] = km2
        m16[:, NSC + B * SKT : NSC + B * SKT + 128] = ident
        m16[:, NSC + B * SKT + 128 :] = woT[128 * c : 128 * (c + 1)]
        blob[c, ROFF:].view(np.float16)[:] = rs16
    return [{"blob": blob[c]} for c in range(NCORES)]


_NC_CACHE = {}


def _get_program():
    if "nc" not in _NC_CACHE:
        _NC_CACHE["nc"] = build_program()
    return _NC_CACHE["nc"]


def kernel(query, key, value, key_mask, query_mask, Wo, bo, _trace=False):
    query = np.asarray(query, dtype=np.float32)
    key = np.asarray(key, dtype=np.float32)
    value = np.asarray(value, dtype=np.float32)
    key_mask = np.asarray(key_mask, dtype=np.int32)
    query_mask = np.asarray(query_mask, dtype=np.int32)
    Wo = np.asarray(Wo, dtype=np.float32)
    bo = np.asarray(bo, dtype=np.float32)

    nc = _get_program()
    in_maps = shard_inputs(query, key, value, key_mask, query_mask, Wo, bo)
    try:
        res = bass_utils.run_bass_kernel_spmd(
            nc, in_maps, core_ids=list(range(NCORES)), trace=_trace
        )
    except ModuleNotFoundError:
        # axon NTFF profile hook unavailable in this container; run untraced
        res = bass_utils.run_bass_kernel_spmd(
            nc, in_maps, core_ids=list(range(NCORES)), trace=False
        )
    kernel.last_results = res

    out = np.empty((B, SQ, D), np.float32)
    for c in range(NCORES):
        ytq = res.results[c]["yq"]  # int8 [128, B, SQ], features 128c..128c+127
        sc = res.results[c]["mx"] * np.float32(1.0 / 127.0)  # [128, 1]
        jsl = slice(128 * c, 128 * (c + 1))
        for g in range(B):
            out[g, :, jsl] = (ytq[:, g, :] * sc).T + bo[jsl]
    return out


# revision 30
# speedup vs baseline: 1.5142x; 1.1163x over previous
"""Trainium2 Bass kernel for nn_CrossAttention (b=2, sq=sk=2048, d=1024, h=16).

Wire-optimized sharding: per-call wall clock is dominated by host<->device
transfer over the axon tunnel (~100MB/s, serialized across cores), so every
tensor is shipped exactly once in fp16 with no cross-core replication:
each of the 8 cores owns 2 heads x both batches (a contiguous 128-column
d-slice of q/k/v in natural [s, d] layout — host does only fp16 casts, all
layout work happens on device). The o_proj weight is d-sharded 8-way and the
partial yT[j, b, q] outputs are summed with an on-device ReduceScatter; each
core downloads its disjoint 128-feature slice quantized to int8 with
per-feature-row scales (rel err ~0.4% vs the 2e-2 budget).

Per-core device pipeline:
  qT/kT tiles  = PE transpose of natural q/k tiles      (PE + DVE copy)
  vn          *= km (key mask, multiplicative)          (DVE)
  scoresT[s,q] = sum_d K[s,d] Q[q,d]                    (PE, fp16 -> f32 psum)
  expT         = exp(0.125 * scoresT)                   (ACT, psum->sbuf fp16)
  avT[0:64,q]  = sum_s vn[s,m] expT[s,q]                (PE, accumulated)
  avT[64,q]    = sum_s km[s] expT[s,q]   (denominator)  (PE, accumulated)
  sc[q]        = rs[q] / (avT[64,q] + eps)              (DVE on 1 partition)
  bc[i,q]      = sc[q]                                  (PE K=1 broadcast)
  outT[...]    = avT[i,q] * bc[i,q]                     (DVE -> fp16)
  y_part       = WoT-slice @ outT                       (PE)
  y_rs         = ReduceScatter_add(y_part, ranks 0..7)  (rank keeps jc==rank)
  yq, mx       = int8 quantize with per-row absmax      (DVE + ACT)

Query-mask / fully-masked rows are zeroed by rs, matching the reference's
nan_to_num semantics. Host adds bo and dequantizes during assembly.
"""

import numpy as np

import concourse.mybir as mybir
import concourse.tile as tile
from concourse import bacc
from concourse import bass_utils

FP16 = mybir.dt.float16
F32 = mybir.dt.float32

# full-problem constants
B, SQ, SK, D, H, HD = 2, 2048, 2048, 1024, 16, 64
NCORES = 8
HLOC = H // NCORES  # 2 heads per core
QBLK = 512  # q columns per psum tile
SKT = SK // 128  # 16 s tiles (same count for q and k)

# per-iteration sk-tile chunking: sizes sum to SKT, tags strictly alternate
# so psum-slot reuse distance stays >= 2 across iteration boundaries
CHUNK_PLAN = [(3, "A"), (3, "B"), (3, "A"), (3, "B"), (2, "A"), (2, "B")]


def build_program():
    nih = B * HLOC  # 4 (batch, local-head) pairs
    nqb = SQ // QBLK  # 4 q blocks
    nj = D // 128  # 8 output-feature chunks (ReduceScatter dim)
    nc = bacc.Bacc(
        "TRN2",
        target_bir_lowering=False,
        debug=False,
        enable_asserts=False,
        num_devices=NCORES,
    )

    # all inputs ship as ONE int8 blob per core (each extra array costs ~7ms
    # of transfer dispatch): qkv int8 | misc16 (fp16, bitcast) | rs (fp16)
    NSC = 3 * B * SKT  # 96 decode-scale columns
    MCOLS = NSC + B * SKT + 128 + D  # scales | km2 | ident | wot
    QOFF = 0
    QBYTES = 3 * B * SQ * 128
    MOFF = QOFF + QBYTES
    MBYTES = 128 * MCOLS * 2
    ROFF = MOFF + MBYTES
    RBYTES = B * SQ * 2
    NBLOB = ROFF + RBYTES
    blob = nc.dram_tensor("blob", [NBLOB], mybir.dt.int8, kind="ExternalInput").ap()
    # single output: int8 y slice + the f32 per-row scale bitcast into the
    # last 4 bytes of each partition row
    yq = nc.dram_tensor(
        "yq", [128, B * SQ + 4], mybir.dt.int8, kind="ExternalOutput"
    ).ap()

    with tile.TileContext(nc) as tc:
        with (
            tc.tile_pool(name="const", bufs=1) as cpool,
            tc.tile_pool(name="exp", bufs=4) as epool,
            tc.tile_pool(name="drain", bufs=2) as dpool,
            tc.tile_pool(name="pA", bufs=1, space="PSUM") as pA,
            tc.tile_pool(name="pB", bufs=1, space="PSUM") as pB,
            tc.tile_pool(name="pacc", bufs=1, space="PSUM") as pacc,
            tc.tile_pool(name="pbc", bufs=1, space="PSUM") as pbc,
            tc.tile_pool(name="dram", bufs=1, space="DRAM") as dram,
        ):
            q8_sb = cpool.tile([128, B, SKT, 128], mybir.dt.int8)
            k8_sb = cpool.tile([128, B, SKT, 128], mybir.dt.int8)
            v8_sb = cpool.tile([128, B, SKT, 128], mybir.dt.int8)
            qn_sb = cpool.tile([128, B, SKT, 128], FP16)
            kn_sb = cpool.tile([128, B, SKT, 128], FP16)
            vn_sb = cpool.tile([128, B, SKT, 128], FP16)
            m16_sb = cpool.tile([128, MCOLS], FP16)
            rs_sb = cpool.tile([1, B * SQ], FP16)
            ones_sb = cpool.tile([1, 64], F32)
            qt_sb = cpool.tile([64, nih, SQ], FP16)
            kt_sb = cpool.tile([64, nih, SK], FP16)
            outT_sb = cpool.tile([128, B, SQ], FP16)

            def sc_col(i, b, t):  # [128, 1] decode-scale column (tensor i)
                j = (i * B + b) * SKT + t
                return m16_sb[:, j : j + 1]

            def km2_col(b, t):  # [128, 1] key-mask column for (batch, sk-tile)
                j = NSC + b * SKT + t
                return m16_sb[:, j : j + 1]

            ident_sb = m16_sb[:, NSC + B * SKT : NSC + B * SKT + 128]
            wot0 = NSC + B * SKT + 128  # wot column base inside m16_sb

            y_part = dram.tile([nj, 128, B, SQ], FP16)
            y_rs = dram.tile([128, B, SQ], FP16)

            for i, dst8 in enumerate((q8_sb, k8_sb, v8_sb)):
                for b in range(B):
                    off = (i * B + b) * SQ * 128
                    nc.sync.dma_start(
                        dst8[:, b],
                        blob[off : off + SQ * 128].rearrange(
                            "(t p d) -> p t d", p=128, d=128
                        ),
                    )
            nc.sync.dma_start(
                m16_sb[:],
                blob[MOFF : MOFF + MBYTES].bitcast(FP16).rearrange(
                    "(p c) -> p c", p=128
                ),
            )
            nc.sync.dma_start(
                rs_sb[:],
                blob[ROFF : ROFF + RBYTES].bitcast(FP16).rearrange(
                    "(o c) -> o c", o=1
                ),
            )
            # ones vector for the denominator-broadcast matmul, built on device
            nc.vector.memzero(ones_sb[:])
            nc.vector.tensor_scalar_add(ones_sb[:], ones_sb[:], 1.0)

            # decode int8 -> fp16 with per-row scales (v's scale has the key
            # mask pre-folded on host, so no separate mask multiply is needed)
            for i, (src, dst) in enumerate(
                ((q8_sb, qn_sb), (k8_sb, kn_sb), (v8_sb, vn_sb))
            ):
                for b in range(B):
                    for t in range(SKT):
                        nc.vector.tensor_tensor(
                            dst[:, b, t, :],
                            src[:, b, t, :],
                            sc_col(i, b, t).to_broadcast((128, 128)),
                            mybir.AluOpType.mult,
                        )

            # PE-transpose natural q/k tiles into [hd, s] operand layout
            tpools = (pacc, pbc)
            ttags = ("acc", "bc")
            idx = 0
            for ih in range(nih):
                b, hl = ih // HLOC, ih % HLOC
                for t in range(SKT):
                    for src, dst in ((qn_sb, qt_sb), (kn_sb, kt_sb)):
                        tp = tpools[idx % 2].tile([64, 128], FP16, tag=ttags[idx % 2])
                        idx += 1
                        nc.tensor.transpose(
                            tp[:], src[:, b, t, hl * 64 : hl * 64 + 64], ident_sb[:]
                        )
                        nc.vector.tensor_copy(dst[:, ih, t * 128 : (t + 1) * 128], tp[:])

            av_pss = {}

            def drain_iter(it):
                ih, qb = it
                b = ih // HLOC
                hl = ih % HLOC
                qsl = slice(qb * QBLK, (qb + 1) * QBLK)
                av_sb = dpool.tile([65, QBLK], F32, tag="avsb")
                nc.vector.tensor_copy(av_sb[:], av_pss[it][:])
                sc = dpool.tile([1, QBLK], F32, tag="sc")
                nc.vector.tensor_scalar_add(sc[:], av_sb[64:65, :], 1e-30)
                nc.vector.reciprocal(sc[:], sc[:])
                nc.vector.tensor_mul(
                    sc[:],
                    sc[:],
                    rs_sb[0:1, b * SQ + qb * QBLK : b * SQ + (qb + 1) * QBLK],
                )
                bc_ps = pbc.tile([64, QBLK], F32, tag="bc")
                nc.tensor.matmul(
                    bc_ps[:], lhsT=ones_sb[:], rhs=sc[:], start=True, stop=True
                )
                nc.vector.tensor_tensor(
                    outT_sb[hl * 64 : hl * 64 + 64, b, qsl],
                    av_sb[0:64, :],
                    bc_ps[:],
                    mybir.AluOpType.mult,
                )

            # flat, software-pipelined chunk stream: QK(c+1) is emitted
            # before AV(c) so the in-order PE queue never waits on exp(c)
            chunks = []
            for ih in range(nih):
                for qb in range(nqb):
                    t0 = 0
                    for csz, tag in CHUNK_PLAN:
                        chunks.append((ih, qb, t0, csz, tag))
                        t0 += csz

            def emit_av(item):
                ih, qb, t0, csz, ex = item
                it = (ih, qb)
                b, hl = ih // HLOC, ih % HLOC
                for j in range(csz):
                    t = t0 + j
                    exsl = ex[:, j * QBLK : (j + 1) * QBLK]
                    nc.tensor.matmul(
                        av_pss[it][0:64, :],
                        lhsT=vn_sb[:, b, t, hl * 64 : hl * 64 + 64],
                        rhs=exsl,
                        start=(t == 0),
                        stop=(t == SKT - 1),
                    )
                    nc.tensor.matmul(
                        av_pss[it][64:65, :],
                        lhsT=km2_col(b, t),
                        rhs=exsl,
                        start=(t == 0),
                        stop=(t == SKT - 1),
                    )
                if t0 + csz == SKT:
                    drain_iter(it)

            pending = []  # depth-2 queue of chunks awaiting AV
            for ci, (ih, qb, t0, csz, tag) in enumerate(chunks):
                it = (ih, qb)
                if t0 == 0:
                    av_pss[it] = pacc.tile(
                        [65, QBLK], F32, tag="acc", name=f"av_ps{ih}_{qb}"
                    )
                pool = pA if tag == "A" else pB
                qk_ps = pool.tile(
                    [128, csz * QBLK], F32, tag="qk" + tag, name=f"qk_ps{ci}"
                )
                qsl = slice(qb * QBLK, (qb + 1) * QBLK)
                for j in range(csz):
                    t = t0 + j
                    nc.tensor.matmul(
                        qk_ps[:, j * QBLK : (j + 1) * QBLK],
                        lhsT=kt_sb[:, ih, t * 128 : (t + 1) * 128],
                        rhs=qt_sb[:, ih, qsl],
                        start=True,
                        stop=True,
                    )
                if len(pending) == 2:
                    emit_av(pending.pop(0))
                ex = epool.tile([128, csz * QBLK], FP16, tag="exp")
                nc.scalar.activation(
                    ex[:], qk_ps[:], mybir.ActivationFunctionType.Exp, scale=0.125
                )
                pending.append((ih, qb, t0, csz, ex))

            for item in pending:
                emit_av(item)

            # partial o-proj: y_part[jc, j, b, q] = sum_p wot[p, jc*128+j] outT[p, b, q]
            for jc in range(nj):
                for b in range(B):
                    for qb in range(nqb):
                        pool = pA if (jc * B * nqb + b * nqb + qb) % 2 == 0 else pB
                        y_ps = pool.tile(
                            [128, QBLK], F32, tag="qk" + ("A" if pool is pA else "B")
                        )
                        qsl = slice(qb * QBLK, (qb + 1) * QBLK)
                        nc.tensor.matmul(
                            y_ps[:],
                            lhsT=m16_sb[:, wot0 + jc * 128 : wot0 + (jc + 1) * 128],
                            rhs=outT_sb[:, b, qsl],
                            start=True,
                            stop=True,
                        )
                        y_sb = dpool.tile([128, QBLK], FP16, tag="y")
                        nc.vector.tensor_copy(y_sb[:], y_ps[:])
                        nc.sync.dma_start(y_part[jc, :, b, qsl], y_sb[:])

            # column-sharded o-proj all-reduce: each rank keeps jc == rank
            nc.gpsimd.collective_compute(
                "ReduceScatter",
                mybir.AluOpType.add,
                replica_groups=[list(range(NCORES))],
                ins=[y_part.opt()],
                outs=[y_rs.opt()],
            )

            # int8 downcast with per-feature-row scales: halves the download
            y_all = cpool.tile([128, B, SQ], FP16)
            nc.sync.dma_start(y_all[:], y_rs[:])
            mx_sb = cpool.tile([128, 1], F32)
            nc.vector.tensor_reduce(
                mx_sb[:],
                y_all[:],
                axis=mybir.AxisListType.XY,
                op=mybir.AluOpType.max,
                apply_absolute_value=True,
            )
            inv_sb = cpool.tile([128, 1], F32)
            nc.vector.tensor_scalar_add(inv_sb[:], mx_sb[:], 1e-30)
            nc.vector.reciprocal(inv_sb[:], inv_sb[:])
            nc.vector.tensor_scalar_mul(inv_sb[:], inv_sb[:], 127.0)
            yq_sb = cpool.tile([128, B, SQ], mybir.dt.int8)
            nc.scalar.activation(
                yq_sb[:], y_all[:], mybir.ActivationFunctionType.Copy, scale=inv_sb[:]
            )
            nc.sync.dma_start(
                yq[:, 0 : B * SQ],
                yq_sb[:].rearrange("p b q -> p (b q)"),
            )
            nc.sync.dma_start(yq[:, B * SQ : B * SQ + 4].bitcast(F32), mx_sb[:])

    nc.compile()
    return nc


def shard_inputs(query, key, value, key_mask, query_mask, Wo, bo):
    """Full inputs -> per-core input maps. Host does only casts + tiny prep;
    all layout transposes happen on device."""
    km01 = key_mask[:, :, 0] != 0  # [B, SK] bool
    qm01 = query_mask[:, :, 0] != 0  # [B, SQ]
    any_km = km01.any(axis=1)  # [B]

    # blob layout must match build_program: qkv int8 | misc16 fp16 | rs fp16
    QBYTES = 3 * B * SQ * 128
    MCOLS = 3 * B * SKT + B * SKT + 128 + D  # scales | km2 | ident | wot
    MBYTES = 128 * MCOLS * 2
    ROFF = QBYTES + MBYTES
    NBLOB = ROFF + B * SQ * 2
    blob = np.empty((NCORES, NBLOB), np.int8)

    # int8 quantize q/k/v with per-(row, 128-col core slice) scales; the key
    # mask is pre-folded into v's decode scale so masked rows decode to 0.
    # qdst aliases the blob so the quantized bytes land per-core directly.
    qdst = blob[:, :QBYTES].reshape(NCORES, 3, B, SQ, 128).transpose(1, 2, 3, 0, 4)
    scales = np.empty((3, B, SQ, NCORES), np.float32)
    tmp = np.empty((B, SQ, NCORES, 128), np.float32)
    for i, x in enumerate((query, key, value)):
        xb = x.reshape(B, SQ, NCORES, 128)
        mx = np.abs(xb).max(axis=-1)  # [B, SQ, NCORES]
        inv = np.float32(127.0) / (mx + np.float32(1e-30))
        np.multiply(xb, inv[..., None], out=tmp)
        np.rint(tmp, out=tmp)
        qdst[i] = tmp
        scales[i] = mx * np.float32(1.0 / 127.0)
    scales[2] *= km01[:, :, None]  # fold key mask into v decode
    # per-core scale columns: [128, 3, B, SKT] with partition = s % 128
    sc_all = np.ascontiguousarray(
        scales.astype(np.float16)
        .reshape(3, B, SKT, 128, NCORES)
        .transpose(4, 3, 0, 1, 2)
    )  # [NCORES, 128, 3, B, SKT]

    km2 = np.ascontiguousarray(
        km01.astype(np.float16).reshape(B, SKT, 128).transpose(2, 0, 1)
    ).reshape(128, B * SKT)
    ident = np.eye(128, dtype=np.float16)
    woT = np.ascontiguousarray(Wo.T).astype(np.float16)  # [d, j]
    rs16 = (qm01 & any_km[:, None]).astype(np.float16).reshape(B * SQ)

    NSC = 3 * B * SKT
    for c in range(NCORES):
        m16 = blob[c, QBYTES:ROFF].view(np.float16).reshape(128, MCOLS)
        m16[:, :NSC] = sc_all[c].reshape(128, NSC)
        m16[:, NSC : NSC + B * SKT] = km2
        m16[:, NSC + B * SKT : NSC + B * SKT + 128] = ident
        m16[:, NSC + B * SKT + 128 :] = woT[128 * c : 128 * (c + 1)]
        blob[c, ROFF:].view(np.float16)[:] = rs16
    return [{"blob": blob[c]} for c in range(NCORES)]


_NC_CACHE = {}


def _get_program():
    if "nc" not in _NC_CACHE:
        _NC_CACHE["nc"] = build_program()
    return _NC_CACHE["nc"]


def kernel(query, key, value, key_mask, query_mask, Wo, bo, _trace=False):
    query = np.asarray(query, dtype=np.float32)
    key = np.asarray(key, dtype=np.float32)
    value = np.asarray(value, dtype=np.float32)
    key_mask = np.asarray(key_mask, dtype=np.int32)
    query_mask = np.asarray(query_mask, dtype=np.int32)
    Wo = np.asarray(Wo, dtype=np.float32)
    bo = np.asarray(bo, dtype=np.float32)

    nc = _get_program()
    in_maps = shard_inputs(query, key, value, key_mask, query_mask, Wo, bo)
    try:
        res = bass_utils.run_bass_kernel_spmd(
            nc, in_maps, core_ids=list(range(NCORES)), trace=_trace
        )
    except ModuleNotFoundError:
        # axon NTFF profile hook unavailable in this container; run untraced
        res = bass_utils.run_bass_kernel_spmd(
            nc, in_maps, core_ids=list(range(NCORES)), trace=False
        )
    kernel.last_results = res

    out = np.empty((B, SQ, D), np.float32)
    for c in range(NCORES):
        raw = res.results[c]["yq"]  # int8 [128, B*SQ + 4]
        ytq = raw[:, : B * SQ].reshape(128, B, SQ)
        mx = np.ascontiguousarray(raw[:, B * SQ :]).view(np.float32)  # [128, 1]
        sc = mx * np.float32(1.0 / 127.0)
        jsl = slice(128 * c, 128 * (c + 1))
        for g in range(B):
            out[g, :, jsl] = (ytq[:, g, :] * sc).T + bo[jsl]
    return out
